# revision 1
# baseline (speedup 1.0000x reference)
"""Blockwise-quant linear (fp8 e4m3fn weights + per-(row,128-block) activation
quant) as a Trainium2 Bass/Tile kernel, row-parallel over 8 NeuronCores.

y[m,n] = sum_k xd[m,k] * wd[n,k], where
  xd = e4m3fn_round(x / a_s) * a_s,  a_s[m,kb] = max(amax128(x), 1e-4)/448
  wd = fp8_weight * w_scale[nb,kb]

Sharding: rows of x (M) split across cores; weight/w_scale replicated.
Each core computes y[1024, 4096] f32; host concatenates.

Device fp8 is IEEE e4m3 (max 240), reference uses e4m3fn (max 448):
 - weight bytes reinterpret exactly (values never reach exp-field-15),
 - activation quant uses half-scale: e4m3fn(v) == 2*e4m3(v/2) for |v|>2^-5.

v3 structure (single fused pipeline):
 - chunk-pair-0 GEMM is software-pipelined one m-tile behind the x-path;
   each slot's GEMM is emitted BEFORE the next x-path so the PE queue is
   [... MM(mt-1) x64, T(mt) x32 ...] and never head-of-line blocks on the
   quant chain.
 - accumulation chains are ch-outer (32 same-bank matmuls per chain) --
   bank ping-pong per matmul triggers the known PE micro-idle/HAM
   oscillation mode and measured 28% slower.
 - weight dequant runs on three engines: DVE / GpSimd tensor_tensor with
   broadcast scales, plus an ACT path using ws_all[:, kb, nb] as a
   per-partition scalar (4x [128,128] activation ops per tile).
"""

import os
from contextlib import ExitStack

import ml_dtypes
import numpy as np

import concourse.bass as bass
import concourse.mybir as mybir
import concourse.tile as tile
from concourse import bacc
from concourse.bass_utils import run_bass_kernel_spmd
from concourse.masks import make_identity

M, K, N = 8192, 4096, 4096
B = 128                 # quant block
NCORES = 8
MS = M // NCORES        # 1024 rows of x per core
KB = K // B             # 32 k-blocks
NB = N // B             # 32 n-blocks
CW = 512                # matmul moving width (1 PSUM bank of f32)
NCH = N // CW           # 8 output column chunks per core
MT = MS // B            # 8 m-tiles per core
G = 4                   # x-path column groups per m-tile
GK = KB // G            # 8 k-blocks per group

F32 = mybir.dt.float32
BF16 = mybir.dt.bfloat16
FP8 = mybir.dt.float8e4


def _drain(nc, pend):
    # xdT drains on ACT (DVE carries amax + quant + dequant)
    pst, dst, g = pend
    nc.scalar.copy(dst, pst.rearrange("p (g j) -> p g j", j=B))


def _kernel_body(tc, nc, x_in, w_in, s_in, y_out):
    with ExitStack() as ctx:
        consts = ctx.enter_context(tc.tile_pool(name="consts", bufs=1))
        xpool = ctx.enter_context(tc.tile_pool(name="xpool", bufs=3))
        spool = ctx.enter_context(tc.tile_pool(name="spool", bufs=2))
        xqpool = ctx.enter_context(tc.tile_pool(name="xqpool", bufs=2))
        xdpool = ctx.enter_context(tc.tile_pool(name="xdpool", bufs=2))
        xdtp = ctx.enter_context(tc.tile_pool(name="xdtp", bufs=1))
        wqpool = ctx.enter_context(tc.tile_pool(name="wqpool", bufs=6))
        wdpool = ctx.enter_context(tc.tile_pool(name="wdpool", bufs=KB + 4))
        ypool = ctx.enter_context(tc.tile_pool(name="ypool", bufs=4))
        psum = ctx.enter_context(tc.tile_pool(name="psum", bufs=1, space="PSUM"))

        identity = consts.tile([B, B], BF16, name="identity")
        make_identity(nc, identity)

        # w_scale, host-expanded to [128, KB, NB] (same value on every partition)
        ws_all = consts.tile([B, KB, NB], F32, name="ws_all")
        nc.gpsimd.dma_start(ws_all[:], s_in[:])

        # resident dequantized-transposed activations: [128(k), kb, MS(m)]
        xdT = xdtp.tile([B, KB, MS], BF16, name="xdT")

        wds = {}

        def emit_w(pair, kb, eng):
            # fused pair-wide dequant: one [128, 2*CW] tile covers both
            # chunks of the pair for this k-block (halves the op count and
            # makes both chunks' weights ready simultaneously)
            nb0 = pair[0] * (CW // B)
            nbw = 2 * CW // B
            wq = wqpool.tile([B, 2 * CW], FP8, name="wq", tag="wq")
            nc.sync.dma_start(wq[:, :CW], w_in[pair[0], kb])
            nc.sync.dma_start(wq[:, CW:], w_in[pair[1], kb])
            wd = wdpool.tile([B, 2 * CW], BF16, name="wd", tag="wd")
            if eng is nc.scalar:
                # ACT path: w_scale[nb,kb] is partition-replicated in ws_all,
                # so ws_all[:, kb, nb] is a valid per-partition scalar operand
                for nb in range(nbw):
                    i = nb0 + nb
                    nc.scalar.mul(
                        wd[:, nb * B : (nb + 1) * B],
                        wq[:, nb * B : (nb + 1) * B],
                        ws_all[:, kb, i : i + 1],
                    )
            else:
                eng.tensor_tensor(
                    wd.rearrange("p (b j) -> p b j", j=B),
                    wq.rearrange("p (b j) -> p b j", j=B),
                    ws_all[:, kb, nb0 : nb0 + nbw].broadcast_to([B, nbw, B]),
                    op=mybir.AluOpType.mult,
                )
            wds[pair[0], kb] = wd[:, :CW]
            wds[pair[1], kb] = wd[:, CW:]

        def emit_w_pair(pair, engs, start=0, count=KB):
            for i in range(start, min(start + count, KB)):
                emit_w(pair, i, engs[i % len(engs)])

        xnats = {}

        def emit_xdma(mt):
            ms = slice(mt * B, (mt + 1) * B)
            xnat = xpool.tile([B, K], BF16, name="xnat", tag="xnat")
            with tc.high_priority():
                for g in range(G):
                    nc.sync.dma_start(
                        xnat[:, g * GK * B : (g + 1) * GK * B],
                        x_in[ms, g * GK * B : (g + 1) * GK * B],
                    )
            xnats[mt] = xnat

        def _scale_chain(xnat, amax, tsc, r2, gk):
            nc.vector.tensor_reduce(
                amax[:, gk],
                xnat[:, gk.start * B : gk.stop * B].rearrange(
                    "p (b j) -> p b j", j=B
                ),
                axis=mybir.AxisListType.X,
                op=mybir.AluOpType.max,
                apply_absolute_value=True,
            )
            # tsc = max(amax, 1e-4)/224  == 2*a_s (half-scale dequant scale)
            nc.vector.tensor_scalar(
                tsc[:, gk], amax[:, gk], 1e-4, 1.0 / 224.0,
                op0=mybir.AluOpType.max, op1=mybir.AluOpType.mult,
            )
            nc.vector.reciprocal(r2[:, gk], tsc[:, gk])

        def emit_xpath(mt):
            # High priority only while filling the pipeline (first two
            # m-tiles): later, wdeq supply for the pair-0 GEMM chains must
            # not be starved by the quant chain (measured: a blanket
            # high-priority x-path costs ~40us of phase-1 PE stalls).
            if mt < 2:
                with tc.high_priority():
                    _emit_xpath(mt)
            else:
                _emit_xpath(mt)

        def _emit_xpath(mt):
            ms = slice(mt * B, (mt + 1) * B)
            xnat = xnats.pop(mt)
            amax = spool.tile([B, KB], F32, name="amax", tag="amax")
            tsc = spool.tile([B, KB], F32, name="tsc", tag="tsc")
            r2 = spool.tile([B, KB], F32, name="r2", tag="r2")
            xq = xqpool.tile([B, K], FP8, name="xq", tag="xq")
            xd = xdpool.tile([B, K], BF16, name="xd", tag="xd")
            if mt > 0:
                # whole-tile scale ops (fewer per-op overheads on DVE)
                _scale_chain(xnat, amax, tsc, r2, slice(0, KB))
            pend = None
            for g in range(G):
                gk = slice(g * GK, (g + 1) * GK)
                gq = slice(g * GK * B, (g + 1) * GK * B)
                if mt == 0:
                    # per-group chain: minimizes time-to-first-transpose
                    _scale_chain(xnat, amax, tsc, r2, gk)
                x3 = xnat[:, gq].rearrange("p (b j) -> p b j", j=B)
                # quantize the whole group in one op on GpSimd; fp8 RTNE on
                # the store (DVE is the phase-1 critical engine: it keeps
                # amax + most of the pair-0 weight dequant)
                nc.gpsimd.tensor_tensor(
                    xq[:, gq].rearrange("p (b j) -> p b j", j=B),
                    x3,
                    r2[:, gk].broadcast_to([B, GK, B]),
                    op=mybir.AluOpType.mult,
                )
                # dequantize the whole group on DVE (fastest engine for the
                # fp8-in/bf16-out broadcast multiply: ~1.4us vs ACT's 3.2)
                nc.vector.tensor_tensor(
                    xd[:, gq].rearrange("p (b j) -> p b j", j=B),
                    xq[:, gq].rearrange("p (b j) -> p b j", j=B),
                    tsc[:, gk].broadcast_to([B, GK, B]),
                    op=mybir.AluOpType.mult,
                )
                # PE-transpose GK k-blocks into one PSUM bank; wide drain copy
                # deferred one group so the engines never head-of-line block
                pst = psum.tile([B, GK * B], BF16, name="pst", tag="pst", bufs=2)
                for j in range(GK):
                    kb = g * GK + j
                    nc.tensor.transpose(
                        pst[:, j * B : (j + 1) * B],
                        xd[:, kb * B : (kb + 1) * B],
                        identity[:],
                    )
                if pend is not None:
                    _drain(nc, pend)
                pend = (pst, xdT[:, gk, ms], g)
            _drain(nc, pend)

        def emit_gemm_ch(ch, mt, lo=0, hi=KB, acc=None):
            ms = slice(mt * B, (mt + 1) * B)
            if acc is None:
                acc = psum.tile([B, CW], F32, name="acc", tag="acc", bufs=6)
            for kb in range(lo, hi):
                nc.tensor.matmul(
                    acc[:],
                    xdT[:, kb, ms],
                    wds[ch, kb],
                    start=(kb == 0),
                    stop=(kb == KB - 1),
                )
            if hi < KB:
                return acc
            yt = ypool.tile([B, CW], F32, name="yt", tag="yt")
            nc.scalar.copy(yt[:], acc[:])
            nc.sync.dma_start(y_out[ms, ch * CW : (ch + 1) * CW], yt[:])
            return None

        def emit_gemm(pair, mt):
            for ch in pair:
                emit_gemm_ch(ch, mt)

        # ---- phase 1: x-path software-pipelined with chunk-pair-0 GEMM.
        # Chunk 0's chains lag the x-path by 2 m-tiles, chunk 1's by 4, so
        # the pair-0 weight dequant (mostly on DVE; front-loaded 6 tiles in
        # the dead time before x lands, then 14/slot, ch-major) stays ahead
        # of the consumption chains. GEMM chains are emitted before
        # xpath(mt) so the PE queue never head-of-line blocks.
        W0 = [nc.vector, nc.gpsimd, nc.scalar, nc.vector, nc.gpsimd,
              nc.vector, nc.scalar]
        emit_xdma(0)
        emit_xdma(1)
        emit_w_pair((0, 1), W0, start=0, count=4)
        for mt in range(MT):
            if mt < 4:
                emit_w_pair((0, 1), W0, start=4 + 7 * mt, count=7)
            if mt >= 3:
                emit_gemm_ch(0, mt - 3)
            if mt >= 4:
                emit_gemm_ch(1, mt - 4)
            emit_xpath(mt)
            if mt + 2 < MT:
                emit_xdma(mt + 2)
        for mt in range(MT - 3, MT):
            emit_gemm_ch(0, mt)
        for mt in range(MT - 4, MT):
            emit_gemm_ch(1, mt)

        # ---- phase 2: remaining chunk pairs; each pair's weight dequant
        # overlaps its own GEMM (supply outruns the consumption chains) ----
        for cp in range(1, NCH // 2):
            pair = (2 * cp, 2 * cp + 1)
            emit_w_pair(pair, [nc.vector, nc.vector, nc.gpsimd, nc.scalar])
            for mt in range(MT):
                emit_gemm(pair, mt)


def build():
    nc = bacc.Bacc(
        "TRN2", target_bir_lowering=False, debug=False, enable_asserts=False
    )
    x_in = nc.dram_tensor("x", (MS, K), BF16, kind="ExternalInput")
    w_in = nc.dram_tensor("wt", (NCH, KB, B, CW), FP8, kind="ExternalInput")
    s_in = nc.dram_tensor("ws", (B, KB, NB), F32, kind="ExternalInput")
    y_out = nc.dram_tensor("y", (MS, N), F32, kind="ExternalOutput")
    with tile.TileContext(nc) as tc:
        _kernel_body(tc, nc, x_in, w_in, s_in, y_out)
    nc.compile()
    return nc


def prep_inputs(x, weight, w_scale):
    """Host-side shard/layout prep. Returns in_maps for the 8 cores."""
    x = np.asarray(x)
    weight = np.asarray(weight)
    w_scale = np.asarray(w_scale, dtype=np.float32)

    # weight bytes reinterpret e4m3fn -> e4m3 exactly iff no exp-field-15 values
    wf = weight.astype(np.float32)
    assert np.abs(wf).max() <= 240.0, "weight has |v|>240; byte reinterpret invalid"
    del wf
    # wt[ch, kb, p, j] = weight[ch*CW + j, kb*B + p]
    w_prep = np.ascontiguousarray(
        weight.T.reshape(KB, B, NCH, CW).transpose(2, 0, 1, 3)
    ).view(ml_dtypes.float8_e4m3)

    # ws[p, kb, nb] = w_scale[nb, kb]
    ws_prep = np.ascontiguousarray(np.broadcast_to(w_scale.T[None], (B, KB, NB)))

    in_maps = []
    for c in range(NCORES):
        in_maps.append(
            {
                "x": np.ascontiguousarray(x[c * MS : (c + 1) * MS]),
                "wt": w_prep,
                "ws": ws_prep,
            }
        )
    return in_maps


_CACHE = {}
LAST_RESULTS = None


def kernel(x, weight, w_scale):
    global LAST_RESULTS
    if "nc" not in _CACHE:
        _CACHE["nc"] = build()
    nc = _CACHE["nc"]
    in_maps = prep_inputs(x, weight, w_scale)
    try:
        res = run_bass_kernel_spmd(
            nc,
            in_maps,
            core_ids=list(range(NCORES)),
            trace=bool(int(os.environ.get("KBQ_TRACE", "0"))),
        )
    except ModuleNotFoundError:
        # tracing unavailable (no NTFF hook module in this image): run plain
        os.environ["BASS_NEVER_TRACE"] = "1"
        res = run_bass_kernel_spmd(
            nc, in_maps, core_ids=list(range(NCORES)), trace=False
        )
    LAST_RESULTS = res
    return np.concatenate([r["y"] for r in res.results], axis=0)

